# revision 18
# baseline (speedup 1.0000x reference)
"""Trainium2 kernel for nn_DDApprox: batched DDOpt (Wilson-Dirac D^dag D) applied
to a fixed basis, over B=256 gauge configs.

Key observation: for each gauge config b, DDOpt is a linear operator on C^128
(L*L*2 = 128 spinor components). With the basis as rows Psi (K,128):

    out_b = Psi @ M_b,   M_b = D_b^T G5 D_b^T G5 = A_b @ A_b,  A_b = D_b^T * g5

D_b is a 5-point stencil matrix built directly from the U(1) links on the host
(cheap: ~9 nonzeros/row). The device then runs a batched real matmul in block
form with output columns interleaved (re,im) so the result views as complex64.

The whole problem is HBM-bandwidth bound (output alone is 134 MB in fp32), so
everything on the wire is bf16: the basis, the M matrices, and the output
(rel tolerance is 2e-2; bf16 keeps us ~100x under it). DMA is batched into a
handful of large transfers (4-pair R loads, 2-pair output stores) and matmuls
are ordered so PE streams back-to-back and ramps to full clock.

Sharding: data-parallel over B across 8 cores (32 configs each); every core
holds the full (small) basis.
"""
import numpy as np
import ml_dtypes

import concourse.bass as bass
import concourse.mybir as mybir
import concourse.tile as tile
from concourse import bacc
from concourse.bass_utils import run_bass_kernel_spmd

N_CORES = 8
B, K, L = 256, 512, 8
KAPPA = 0.276
B_PER_CORE = B // N_CORES
N_PAIR = B_PER_CORE // 2   # 2 configs share a matmul free dim
N_GRP = N_PAIR // 4        # 4 pairs share one R load
N_ST = N_PAIR // 2         # 2 pairs share one output store

_G0 = np.array([[0, 1], [1, 0]], np.complex64)
_G1 = np.array([[0, -1j], [1j, 0]], np.complex64)


def _build_M(u1_real, u1_imag):
    """Dense DDOpt^T matrices: M_b such that out_b = Psi @ M_b."""
    U = (u1_real + 1j * u1_imag).astype(np.complex64)  # (B,2,L,L)
    Bn = U.shape[0]
    n = 2 * L * L
    D = np.zeros((Bn, n, n), np.complex64)
    idx = np.arange(n)
    D[:, idx, idx] = 1.0

    x, y = np.meshgrid(np.arange(L), np.arange(L), indexing="ij")
    site = (x * L + y).ravel()
    xp = ((x + 1) % L * L + y).ravel()
    xm = ((x - 1) % L * L + y).ravel()
    yp = (x * L + (y + 1) % L).ravel()
    ym = (x * L + (y - 1) % L).ravel()
    s = np.arange(2)

    def scatter(nbr_site, P, coeff):
        rows = np.broadcast_to(site[:, None, None] * 2 + s[None, :, None], (64, 2, 2)).ravel()
        cols = np.broadcast_to(nbr_site[:, None, None] * 2 + s[None, None, :], (64, 2, 2)).ravel()
        vals = (coeff[:, :, None, None] * P[None, None, :, :]).reshape(Bn, -1)
        D[:, rows, cols] += -KAPPA * vals

    U0 = U[:, 0].reshape(Bn, -1)
    U1 = U[:, 1].reshape(Bn, -1)
    I2 = np.eye(2, dtype=np.complex64)
    scatter(xp, I2 - _G0, U0)
    scatter(xm, I2 + _G0, np.conj(U0[:, xm]))
    scatter(yp, I2 - _G1, U1)
    scatter(ym, I2 + _G1, np.conj(U1[:, ym]))

    g5v = np.tile(np.array([1.0, -1.0], np.float32), L * L)
    A = D.transpose(0, 2, 1) * g5v[None, None, :]
    return (A @ A).astype(np.complex64)


def _build_device_inputs(u1_real, u1_imag, basis_real, basis_imag):
    """psit (128,2,K) bf16 and R (B/8, 128, 4, 512) bf16 grouped 4 pairs/load."""
    M = _build_M(u1_real, u1_imag)
    Bn = M.shape[0]
    Mr, Mi = M.real.astype(np.float32), M.imag.astype(np.float32)
    # Only the top block row [Mr | Mi] (interleaved) ships to the device; the
    # bottom row [-Mi | Mr] is its column swap/negate, built on-chip.
    R = np.empty((Bn, 128, 256), np.float32)
    R[:, :, 0::2] = Mr
    R[:, :, 1::2] = Mi
    PsiT = np.concatenate(
        [basis_real.reshape(K, 128).T, basis_imag.reshape(K, 128).T], axis=0
    ).astype(np.float32)
    # psit_dev (128,2,512): [p,c,kt*128+j] = PsiT[c*128+p, j*4+kt]
    # (k interleaved so psum tile kt holds k = p*4+kt -> out rows land in
    #  natural k order without any host-side gather)
    PsiT_perm = PsiT.reshape(256, 128, 4).transpose(0, 2, 1).reshape(256, K)
    psit_dev = np.ascontiguousarray(PsiT_perm.reshape(2, 128, K).transpose(1, 0, 2))
    # Pair consecutive configs along the matmul free dim: (Bn/2,128,512)
    # [pair,p,:256] = R[2*pair, p, :], [pair,p,256:] = R[2*pair+1, p, :]
    Rp = R.reshape(Bn // 2, 2, 128, 256).transpose(0, 2, 1, 3).reshape(Bn // 2, 128, 512)
    # Group 4 pairs per DMA: (Bn/8, 128, 4, 512), per-partition 4KB contiguous
    Rg = np.ascontiguousarray(Rp.reshape(Bn // 8, 4, 128, 512).transpose(0, 2, 1, 3))
    # Second block row [-Mi | Mr] for each core's FIRST group, shipped from the
    # host so group 0's matmuls don't wait on the on-device build (head latency)
    Rg0 = Rg[::4]  # (n_cores, 128, 4, 512) when Bn=256
    R1h = np.empty_like(Rg0)
    R1h[..., 0::2] = -Rg0[..., 1::2]
    R1h[..., 1::2] = Rg0[..., 0::2]
    return (
        np.ascontiguousarray(psit_dev).astype(ml_dtypes.bfloat16),
        Rg.astype(ml_dtypes.bfloat16),
        np.ascontiguousarray(R1h).astype(ml_dtypes.bfloat16),
    )


def _build_nc(n_b, mm_dt=mybir.dt.bfloat16):
    """Per-core kernel: out[b] (K,256) = PsiT.T (K,256c) @ R[b] (256c,256)."""
    nc = bacc.Bacc(None, target_bir_lowering=False)
    n_pair = n_b // 2
    n_grp = n_pair // 4
    n_st = n_pair // 2
    psit = nc.dram_tensor("psit", [128, 2, K], mm_dt, kind="ExternalInput")
    r = nc.dram_tensor("r", [n_grp, 128, 4, 512], mm_dt, kind="ExternalInput")
    r1h = nc.dram_tensor("r1h", [128, 4, 512], mm_dt, kind="ExternalInput")
    # out[g, p, q, t, 0:256] = config 2*(4*g+q), [...,256:512] = +1,
    # rows k = p*4 + t; host de-interleaves the b-pair axis.
    out = nc.dram_tensor("out", [n_grp, 128, 4, K // 128, 512], mm_dt, kind="ExternalOutput")

    f32 = mybir.dt.float32

    with tile.TileContext(nc) as tc:
        with (
            tc.tile_pool(name="singles", bufs=1) as singles,
            tc.tile_pool(name="rpool", bufs=4) as rpool,
            tc.tile_pool(name="opool", bufs=2) as opool,
            tc.tile_pool(name="psum", bufs=4, space="PSUM") as psum_pool,
        ):
            psit_sb = singles.tile([128, 2, K], mm_dt)
            r1h_sb = singles.tile([128, 4, 512], mm_dt)
            # psit on gpsimd, group-0's host-built second block row on scalar,
            # r prefetches alone on sync: nothing ever queues ahead of an r load
            nc.gpsimd.dma_start(out=psit_sb[:], in_=psit[:])
            nc.scalar.dma_start(out=r1h_sb[:], in_=r1h[:])
            for g in range(n_grp):
                r_sb = rpool.tile([128, 2, 4, 512], mm_dt)
                nc.sync.dma_start(out=r_sb[:, 0], in_=r[g])
                if g == 0:
                    rhs_c = [r_sb[:, 0], r1h_sb]
                else:
                    # block row c=1 is [-Mi | Mr]: swap (re,im) pairs, negate re
                    r0 = r_sb[:, 0].rearrange("p q (n two) -> p q n two", two=2)
                    r1 = r_sb[:, 1].rearrange("p q (n two) -> p q n two", two=2)
                    nc.vector.tensor_copy(r1[:, :, :, 1], r0[:, :, :, 0])
                    nc.scalar.mul(r1[:, :, :, 0], r0[:, :, :, 1], -1.0)
                    rhs_c = [r_sb[:, 0], r_sb[:, 1]]
                otile = opool.tile([128, 4, 4, 512], mm_dt, name="o_sb")
                for half in range(2):
                    for kt in (2 * half, 2 * half + 1):
                        # two 2-bank psum tiles per kt (q01, q23): 4-tile pool
                        # rotation = all 8 banks, overlaps halves with copies
                        pq = [
                            psum_pool.tile([128, 2, 512], f32, name="ps")
                            for _ in range(2)
                        ]
                        for c in range(2):
                            for q in range(4):
                                nc.tensor.matmul(
                                    pq[q // 2][:, q % 2, :],
                                    psit_sb[:, c, kt * 128:(kt + 1) * 128],
                                    rhs_c[c][:, q, :],
                                    start=(c == 0), stop=(c == 1),
                                )
                        # wide copies: 2 banks -> [q-pair, kt] slice of otile
                        nc.scalar.copy(otile[:, 0:2, kt, :], pq[0][:])
                        nc.vector.tensor_copy(otile[:, 2:4, kt, :], pq[1][:])
                        # ship each kt as soon as its two copies land; alternate
                        # queues so the tail pair drains in parallel
                        st_eng = nc.gpsimd if kt % 2 == 0 else nc.sync
                        st_eng.dma_start(
                            out=out[g][:, :, kt, :], in_=otile[:, :, kt, :]
                        )
    nc.compile()
    return nc


def kernel(u1_real, u1_imag, basis_real, basis_imag, _want_results_obj=False, _trace=False):
    u1_real = np.asarray(u1_real, np.float32)
    u1_imag = np.asarray(u1_imag, np.float32)
    basis_real = np.asarray(basis_real, np.float32)
    basis_imag = np.asarray(basis_imag, np.float32)

    PsiT, R, R1h = _build_device_inputs(u1_real, u1_imag, basis_real, basis_imag)
    nc = _build_nc(B_PER_CORE)
    n_grp_core = N_GRP
    in_maps = [
        {
            "psit": PsiT,
            "r": np.ascontiguousarray(R[i * n_grp_core:(i + 1) * n_grp_core]),
            "r1h": R1h[i],
        }
        for i in range(N_CORES)
    ]
    res = run_bass_kernel_spmd(nc, in_maps, core_ids=list(range(N_CORES)), trace=_trace)
    full = np.concatenate(
        [np.asarray(res.results[i]["out"], np.float32) for i in range(N_CORES)], axis=0
    )  # (B/8, 128, 4, 4, 512): [g, p, q, t, cfg*256+col], k = p*4 + t
    full = full.transpose(0, 2, 1, 3, 4).reshape(B // 8, 4, K, 2, 256)
    full = full.transpose(0, 1, 3, 2, 4)  # (g, q, cfg, k, col); b = 8*g+2*q+cfg
    out = np.ascontiguousarray(full).reshape(B, K, 256).view(np.complex64)  # (B,K,128)
    if _want_results_obj:
        return out, res
    return out


# revision 23
# speedup vs baseline: 1.1314x; 1.1314x over previous
"""Trainium2 kernel for nn_DDApprox: batched DDOpt (Wilson-Dirac D^dag D) applied
to a fixed basis, over B=256 gauge configs.

Key observation: for each gauge config b, DDOpt is a linear operator on C^128
(L*L*2 = 128 spinor components). With the basis as rows Psi (K,128):

    out_b = Psi @ M_b,   M_b = D_b^T G5 D_b^T G5 = A_b @ A_b,  A_b = D_b^T * g5

D_b is a 5-point stencil matrix built directly from the U(1) links on the host
(cheap: ~9 nonzeros/row). The device then runs a batched real matmul in block
form with output columns interleaved (re,im) so the result views as complex64.

The whole problem is HBM-bandwidth bound (output alone is 134 MB in fp32), so
everything on the wire is bf16: the basis, the M matrices, and the output
(rel tolerance is 2e-2; bf16 keeps us ~100x under it). DMA is batched into a
handful of large transfers (4-pair R loads, 2-pair output stores) and matmuls
are ordered so PE streams back-to-back and ramps to full clock.

Sharding: data-parallel over B across 8 cores (32 configs each); every core
holds the full (small) basis.
"""
import numpy as np
import ml_dtypes

import concourse.bass as bass
import concourse.mybir as mybir
import concourse.tile as tile
from concourse import bacc
from concourse.bass_utils import run_bass_kernel_spmd

N_CORES = 8
B, K, L = 256, 512, 8
KAPPA = 0.276
B_PER_CORE = B // N_CORES
N_PAIR = B_PER_CORE // 2   # 2 configs share a matmul free dim
N_GRP = N_PAIR // 4        # 4 pairs share one R load
N_ST = N_PAIR // 2         # 2 pairs share one output store

_G0 = np.array([[0, 1], [1, 0]], np.complex64)
_G1 = np.array([[0, -1j], [1j, 0]], np.complex64)


def _build_M(u1_real, u1_imag):
    """Dense DDOpt^T matrices: M_b such that out_b = Psi @ M_b."""
    U = (u1_real + 1j * u1_imag).astype(np.complex64)  # (B,2,L,L)
    Bn = U.shape[0]
    n = 2 * L * L
    D = np.zeros((Bn, n, n), np.complex64)
    idx = np.arange(n)
    D[:, idx, idx] = 1.0

    x, y = np.meshgrid(np.arange(L), np.arange(L), indexing="ij")
    site = (x * L + y).ravel()
    xp = ((x + 1) % L * L + y).ravel()
    xm = ((x - 1) % L * L + y).ravel()
    yp = (x * L + (y + 1) % L).ravel()
    ym = (x * L + (y - 1) % L).ravel()
    s = np.arange(2)

    def scatter(nbr_site, P, coeff):
        rows = np.broadcast_to(site[:, None, None] * 2 + s[None, :, None], (64, 2, 2)).ravel()
        cols = np.broadcast_to(nbr_site[:, None, None] * 2 + s[None, None, :], (64, 2, 2)).ravel()
        vals = (coeff[:, :, None, None] * P[None, None, :, :]).reshape(Bn, -1)
        D[:, rows, cols] += -KAPPA * vals

    U0 = U[:, 0].reshape(Bn, -1)
    U1 = U[:, 1].reshape(Bn, -1)
    I2 = np.eye(2, dtype=np.complex64)
    scatter(xp, I2 - _G0, U0)
    scatter(xm, I2 + _G0, np.conj(U0[:, xm]))
    scatter(yp, I2 - _G1, U1)
    scatter(ym, I2 + _G1, np.conj(U1[:, ym]))

    g5v = np.tile(np.array([1.0, -1.0], np.float32), L * L)
    A = D.transpose(0, 2, 1) * g5v[None, None, :]
    return (A @ A).astype(np.complex64)


def _build_device_inputs(u1_real, u1_imag, basis_real, basis_imag):
    """psit (128,2,K) bf16 and R (B/8, 128, 4, 512) bf16 grouped 4 pairs/load."""
    M = _build_M(u1_real, u1_imag)
    Bn = M.shape[0]
    Mr, Mi = M.real.astype(np.float32), M.imag.astype(np.float32)
    # Only the top block row [Mr | Mi] (interleaved) ships to the device; the
    # bottom row [-Mi | Mr] is its column swap/negate, built on-chip.
    R = np.empty((Bn, 128, 256), np.float32)
    R[:, :, 0::2] = Mr
    R[:, :, 1::2] = Mi
    PsiT = np.concatenate(
        [basis_real.reshape(K, 128).T, basis_imag.reshape(K, 128).T], axis=0
    ).astype(np.float32)
    # psit_dev (128,2,512): [p,c,kt*128+j] = PsiT[c*128+p, j*4+kt]
    # (k interleaved so psum tile kt holds k = p*4+kt -> out rows land in
    #  natural k order without any host-side gather)
    PsiT_perm = PsiT.reshape(256, 128, 4).transpose(0, 2, 1).reshape(256, K)
    psit_dev = np.ascontiguousarray(PsiT_perm.reshape(2, 128, K).transpose(1, 0, 2))
    # Pair consecutive configs along the matmul free dim: (Bn/2,128,512)
    # [pair,p,:256] = R[2*pair, p, :], [pair,p,256:] = R[2*pair+1, p, :]
    Rp = R.reshape(Bn // 2, 2, 128, 256).transpose(0, 2, 1, 3).reshape(Bn // 2, 128, 512)
    # Group 4 pairs per DMA: (Bn/8, 128, 4, 512), per-partition 4KB contiguous
    Rg = np.ascontiguousarray(Rp.reshape(Bn // 8, 4, 128, 512).transpose(0, 2, 1, 3))

    return (
        np.ascontiguousarray(psit_dev).astype(ml_dtypes.bfloat16),
        Rg.astype(ml_dtypes.bfloat16),
    )


def _build_nc(n_b, mm_dt=mybir.dt.bfloat16):
    """Per-core kernel: out[b] (K,256) = PsiT.T (K,256c) @ R[b] (256c,256)."""
    nc = bacc.Bacc(None, target_bir_lowering=False)
    n_pair = n_b // 2
    n_grp = n_pair // 4
    n_st = n_pair // 2
    psit = nc.dram_tensor("psit", [128, 2, K], mm_dt, kind="ExternalInput")
    r = nc.dram_tensor("r", [n_grp, 128, 4, 512], mm_dt, kind="ExternalInput")
    # out[g, p, q, t, 0:256] = config 2*(4*g+q), [...,256:512] = +1,
    # rows k = p*4 + t; host de-interleaves the b-pair axis.
    out = nc.dram_tensor("out", [n_grp, 128, 4, K // 128, 512], mm_dt, kind="ExternalOutput")

    f32 = mybir.dt.float32

    with tile.TileContext(nc) as tc:
        with (
            tc.tile_pool(name="singles", bufs=1) as singles,
            tc.tile_pool(name="rpool", bufs=4) as rpool,
            tc.tile_pool(name="opool", bufs=2) as opool,
            tc.tile_pool(name="psum", bufs=4, space="PSUM") as psum_pool,
        ):
            psit_sb = singles.tile([128, 2, K], mm_dt)
            # all loads on sync, in wire-priority order: psit feeds the first
            # LDWEIGHTS, then group 0's q01 slice feeds the first matmuls
            nc.sync.dma_start(out=psit_sb[:], in_=psit[:])
            for g in range(n_grp):
                r_sb = rpool.tile([128, 2, 4, 512], mm_dt)
                r0 = r_sb[:, 0].rearrange("p q (n two) -> p q n two", two=2)
                r1 = r_sb[:, 1].rearrange("p q (n two) -> p q n two", two=2)
                if g == 0:
                    # split load + split build: first matmuls start after the
                    # 0.25 MB q01 slice, not the full group
                    for qh in range(2):
                        sl = slice(2 * qh, 2 * qh + 2)
                        nc.sync.dma_start(out=r_sb[:, 0, sl], in_=r[g][:, sl])
                        nc.vector.tensor_copy(r1[:, sl, :, 1], r0[:, sl, :, 0])
                        nc.scalar.mul(r1[:, sl, :, 0], r0[:, sl, :, 1], -1.0)
                else:
                    # block row c=1 is [-Mi | Mr]: swap (re,im) pairs, negate re
                    nc.sync.dma_start(out=r_sb[:, 0], in_=r[g])
                    nc.vector.tensor_copy(r1[:, :, :, 1], r0[:, :, :, 0])
                    nc.scalar.mul(r1[:, :, :, 0], r0[:, :, :, 1], -1.0)
                otile = opool.tile([128, 4, 4, 512], mm_dt, name="o_sb")
                for half in range(2):
                    kts = (2 * half, 2 * half + 1)
                    for kt in kts:
                        # two 2-bank psum tiles per kt (q01, q23): 4-tile pool
                        # rotation = all 8 banks, overlaps halves with copies
                        pq = [
                            psum_pool.tile([128, 2, 512], f32, name="ps")
                            for _ in range(2)
                        ]
                        for c in range(2):
                            for q in range(4):
                                nc.tensor.matmul(
                                    pq[q // 2][:, q % 2, :],
                                    psit_sb[:, c, kt * 128:(kt + 1) * 128],
                                    r_sb[:, c, q, :],
                                    start=(c == 0), stop=(c == 1),
                                )
                        # wide copies: 2 banks -> [q-pair, kt] slice of otile
                        nc.scalar.copy(otile[:, 0:2, kt, :], pq[0][:])
                        nc.vector.tensor_copy(otile[:, 2:4, kt, :], pq[1][:])
                    if g == n_grp - 1 and half == 1:
                        # tail: drain the last two kt slices on separate queues
                        nc.gpsimd.dma_start(out=out[g][:, :, 2, :], in_=otile[:, :, 2, :])
                        nc.sync.dma_start(out=out[g][:, :, 3, :], in_=otile[:, :, 3, :])
                    else:
                        # ship each half as soon as its copies land
                        nc.gpsimd.dma_start(
                            out=out[g][:, :, kts[0]:kts[1] + 1, :],
                            in_=otile[:, :, kts[0]:kts[1] + 1, :],
                        )
    nc.compile()
    return nc


def kernel(u1_real, u1_imag, basis_real, basis_imag, _want_results_obj=False, _trace=False):
    u1_real = np.asarray(u1_real, np.float32)
    u1_imag = np.asarray(u1_imag, np.float32)
    basis_real = np.asarray(basis_real, np.float32)
    basis_imag = np.asarray(basis_imag, np.float32)

    PsiT, R = _build_device_inputs(u1_real, u1_imag, basis_real, basis_imag)
    nc = _build_nc(B_PER_CORE)
    n_grp_core = N_GRP
    in_maps = [
        {"psit": PsiT, "r": np.ascontiguousarray(R[i * n_grp_core:(i + 1) * n_grp_core])}
        for i in range(N_CORES)
    ]
    res = run_bass_kernel_spmd(nc, in_maps, core_ids=list(range(N_CORES)), trace=_trace)
    full = np.concatenate(
        [np.asarray(res.results[i]["out"], np.float32) for i in range(N_CORES)], axis=0
    )  # (B/8, 128, 4, 4, 512): [g, p, q, t, cfg*256+col], k = p*4 + t
    full = full.transpose(0, 2, 1, 3, 4).reshape(B // 8, 4, K, 2, 256)
    full = full.transpose(0, 1, 3, 2, 4)  # (g, q, cfg, k, col); b = 8*g+2*q+cfg
    out = np.ascontiguousarray(full).reshape(B, K, 256).view(np.complex64)  # (B,K,128)
    if _want_results_obj:
        return out, res
    return out
